# revision 18
# baseline (speedup 1.0000x reference)
"""Trainium2 Bass kernel for nn_BayesianLSTM (2-layer LSTM H=1024, conv front-end,
84 encoder + 23 decoder steps, B=256, fp32).

Sharding: 2 data-parallel groups (batch 128 each) x 4-way model parallel within
each group (each core owns a 256-wide slice of the hidden dim of both layers).
Recurrent matmuls are batch-stationary (stationary = h^T tile [K=128, B=128],
moving = weight chunks [K=128, N=512]), giving full PE-array utilization.
Hidden-state chunks are exchanged once per layer per step with an intra-group
AllGather of the transposed chunk (h^T layout feeds the next step's stationary
operand directly). Dropout masks (fixed seed, data-independent) are precomputed
on host and folded with fc_w; the fc dot-product partials ride the layer-1
AllGather.
"""
import numpy as np

H = 1024
B_FULL = 256
B = 128           # batch per core (per DP group)
HC = 256          # hidden chunk per core
WIN_IN, WIN_OUT = 168, 24
N_ENC = 84
N_STEPS = N_ENC + WIN_OUT - 1   # 107 cell evaluations
KEEP = 0.8
FP32 = None       # set after imports


def _build(conv_w, conv_b, fc_b, n_steps=N_STEPS, debug_step=None, no_fc=False):
    import concourse.bass as bass
    import concourse.tile as tile
    from concourse import bacc, mybir

    FP = mybir.dt.float32
    AF = mybir.ActivationFunctionType
    OP = mybir.AluOpType

    nc = bacc.Bacc(None, target_bir_lowering=False, num_devices=8)

    # ---- per-core external tensors ----
    x_d = nc.dram_tensor("x", [B, WIN_IN, 8], FP, kind="ExternalInput")
    w0_d = nc.dram_tensor("w0", [H, 1024], FP, kind="ExternalInput")     # Whh0^T slice
    xb0_d = nc.dram_tensor("xb0", [2, 1024], FP, kind="ExternalInput")   # [Wih0 col; b0]
    w1_d = nc.dram_tensor("w1", [2 * H, 1024], FP, kind="ExternalInput") # [Wih1^T; Whh1^T]
    b1_d = nc.dram_tensor("b1", [1, 1024], FP, kind="ExternalInput")
    mask_d = nc.dram_tensor("mask", [24, B, HC], FP, kind="ExternalInput")
    eye_d = nc.dram_tensor("eye", [128, 128], FP, kind="ExternalInput")
    out_d = nc.dram_tensor("out", [24, B], FP, kind="ExternalOutput")
    if debug_step is not None:
        dbg_d = nc.dram_tensor("dbg", [6, B, HC], FP, kind="ExternalOutput")

    GROUPS = [[0, 1, 2, 3], [4, 5, 6, 7]]

    with tile.TileContext(nc) as tc:
        with tc.tile_pool(name="const", bufs=1) as constp, \
             tc.tile_pool(name="work", bufs=2) as work, \
             tc.tile_pool(name="xbp", bufs=3) as xbp, \
             tc.tile_pool(name="stg", bufs=6) as stg, \
             tc.tile_pool(name="ps", bufs=1, space="PSUM") as ps, \
             tc.tile_pool(name="tpp", bufs=4, space="PSUM") as tpp, \
             tc.tile_pool(name="dram", bufs=2, space="DRAM") as dram:

            # ---- load constants ----
            w0 = constp.tile([128, 8, 1024], FP, tag="w0")
            for k in range(8):
                nc.sync.dma_start(w0[:, k, :], w0_d[128 * k:128 * (k + 1), :])
            w1 = constp.tile([128, 16, 1024], FP, tag="w1")
            for k in range(16):
                nc.sync.dma_start(w1[:, k, :], w1_d[128 * k:128 * (k + 1), :])
            xb0w = constp.tile([2, 1024], FP, tag="xb0w")
            nc.sync.dma_start(xb0w[:], xb0_d[:])
            b1 = constp.tile([1, 1024], FP, tag="b1")
            nc.sync.dma_start(b1[:], b1_d[:])
            mask = constp.tile([128, 24, HC], FP, tag="mask")
            for m in range(24):
                nc.sync.dma_start(mask[:, m, :], mask_d[m])
            eye = constp.tile([128, 128], FP, tag="eye")
            nc.sync.dma_start(eye[:], eye_d[:])
            ones = constp.tile([1, 128], FP, tag="ones")
            nc.vector.memset(ones[:], 1.0)

            # ---- conv front-end (per-core batch half) ----
            xpad = constp.tile([128, WIN_IN + 2, 8], FP, tag="xpad")
            nc.vector.memset(xpad[:], 0.0)
            nc.sync.dma_start(xpad[:, 1:WIN_IN + 1, :], x_d[:])
            acc = constp.tile([128, WIN_IN], FP, tag="cacc")
            tmpc = constp.tile([128, WIN_IN], FP, tag="ctmp")
            first = True
            for f in range(8):
                for k in range(3):
                    wv = float(conv_w[0, f, k])
                    src = xpad[:, k:k + WIN_IN, f]
                    if first:
                        nc.vector.tensor_scalar(acc[:], src, wv, None, OP.mult)
                        first = False
                    else:
                        nc.vector.tensor_scalar(tmpc[:], src, wv, None, OP.mult)
                        nc.vector.tensor_add(acc[:], acc[:], tmpc[:])
            relu = constp.tile([128, WIN_IN], FP, tag="crelu")
            nc.scalar.activation(relu[:], acc[:], AF.Relu, bias=float(conv_b[0]))
            pool = constp.tile([128, N_ENC], FP, tag="cpool")
            nc.vector.tensor_max(pool[:], relu[:, 0:WIN_IN:2], relu[:, 1:WIN_IN:2])
            seq_ps = tpp.tile([128, 128], FP, tag="tp")
            nc.tensor.transpose(seq_ps[0:N_ENC, :], pool[:], eye[:])
            seqT = constp.tile([N_ENC, 128], FP, tag="seqT")
            nc.vector.tensor_copy(seqT[:], seq_ps[0:N_ENC, :])

            # ---- recurrent steps ----
            h0T = h1T = None          # [128, 8, 128] stationary h^T tiles
            c0 = c1 = None            # [128, 256]
            agout2_prev = None
            y_last = None

            for t in range(n_steps):
                fc_m = t - (N_ENC - 1)   # mask/output index produced at THIS step's tail
                dec_m = t - N_ENC        # y index consumed at this step (decoder)

                # -- xb stationary tile [2,128]: row0 = x_t (enc) or y (dec), row1 = 1
                xb = xbp.tile([2, 128], FP, tag="xb")
                nc.vector.memset(xb[:], 1.0)
                if t < N_ENC:
                    nc.sync.dma_start(xb[0:1, :], seqT[t:t + 1, :])
                else:
                    tmpy = work.tile([1, 4, 128], FP, tag="tmpy")
                    for r in range(4):
                        nc.sync.dma_start(tmpy[0:1, r, :],
                                          agout2_prev[257 * r + 256:257 * r + 257, :])
                    ya = work.tile([1, 128], FP, tag="ya")
                    nc.vector.tensor_add(ya[:], tmpy[0:1, 0, :], tmpy[0:1, 1, :])
                    nc.vector.tensor_add(ya[:], ya[:], tmpy[0:1, 2, :])
                    nc.vector.tensor_add(ya[:], ya[:], tmpy[0:1, 3, :])
                    nc.vector.tensor_scalar(xb[0:1, :], ya[:], float(fc_b), None, OP.add)
                    nc.sync.dma_start(out_d[dec_m:dec_m + 1, :], xb[0:1, :])

                # PE queue is in-order, so emission order = execution order.
                # Encoder: emit the h0new transposes BEFORE the Whh1 block so
                # AllGather#1 launches ~4us earlier; Whh1 then covers the
                # gather. Decoder: emit Whh1 right after the g0 h-part so the
                # PE has cover while the y-feedback gather (AG#2 of t-1)
                # completes; the xb matmul (which waits on it) comes after.
                g0 = ps.tile([128, 1024], FP, tag="g0")
                g1 = ps.tile([128, 1024], FP, tag="g1")

                def g0_h_part():
                    if t > 0:
                        for k in range(8):
                            for c in range(2):
                                nc.tensor.matmul(g0[:, 512 * c:512 * (c + 1)],
                                                 h0T[:, k, :],
                                                 w0[:, k, 512 * c:512 * (c + 1)],
                                                 start=(k == 0), stop=False)

                def g1_bias_whh1():
                    for c in range(2):
                        nc.tensor.matmul(g1[:, 512 * c:512 * (c + 1)], ones[:],
                                         b1[:, 512 * c:512 * (c + 1)],
                                         start=True, stop=False)
                    if t > 0:
                        for k in range(8):
                            for c in range(2):
                                nc.tensor.matmul(g1[:, 512 * c:512 * (c + 1)],
                                                 h1T[:, k, :],
                                                 w1[:, 8 + k, 512 * c:512 * (c + 1)],
                                                 start=False, stop=False)

                def g0_xb_part():
                    for c in range(2):
                        nc.tensor.matmul(g0[:, 512 * c:512 * (c + 1)], xb[:],
                                         xb0w[:, 512 * c:512 * (c + 1)],
                                         start=(t == 0), stop=True)

                enc_order = t < N_ENC
                g0_h_part()
                if not enc_order:
                    g1_bias_whh1()
                g0_xb_part()

                # -- layer0 activations, cell update
                acts0 = work.tile([128, 1024], FP, tag="acts0")
                nc.scalar.activation(acts0[:, 0:512], g0[:, 0:512], AF.Sigmoid)
                nc.scalar.activation(acts0[:, 512:768], g0[:, 512:768], AF.Tanh)
                nc.scalar.activation(acts0[:, 768:1024], g0[:, 768:1024], AF.Sigmoid)
                c0n = work.tile([128, HC], FP, tag="c0")
                if t == 0:
                    nc.vector.tensor_mul(c0n[:], acts0[:, 0:256], acts0[:, 512:768])
                else:
                    tmp0 = work.tile([128, HC], FP, tag="tmp0")
                    nc.vector.tensor_mul(tmp0[:], acts0[:, 0:256], acts0[:, 512:768])
                    nc.vector.tensor_mul(c0n[:], acts0[:, 256:512], c0[:])
                    nc.vector.tensor_add(c0n[:], c0n[:], tmp0[:])
                c0 = c0n
                tc0 = work.tile([128, HC], FP, tag="tc0")
                nc.scalar.activation(tc0[:], c0[:], AF.Tanh)
                h0new = work.tile([128, HC], FP, tag="h0new")
                nc.vector.tensor_mul(h0new[:], acts0[:, 768:1024], tc0[:])

                # -- transpose h0new chunk, AllGather within group
                agin1 = dram.tile([2, 128, 128], FP, tag="agin1")
                for s in range(2):
                    tp = tpp.tile([128, 128], FP, tag="tp")
                    nc.tensor.transpose(tp[:], h0new[:, 128 * s:128 * (s + 1)], eye[:])
                    st = stg.tile([128, 128], FP, tag="stage")
                    nc.vector.tensor_copy(st[:], tp[:])
                    nc.sync.dma_start(agin1[s], st[:])
                agout1 = dram.tile([8, 128, 128], FP, tag="agout1")
                nc.gpsimd.collective_compute(
                    "AllGather", OP.bypass, replica_groups=GROUPS,
                    ins=[agin1[:]], outs=[agout1[:]])
                if enc_order:
                    g1_bias_whh1()
                h0T = work.tile([128, 8, 128], FP, tag="h0T")
                for k in range(8):
                    nc.sync.dma_start(h0T[:, k, :], agout1[k])

                # -- layer1 gates part 2: h0new @ Wih1^T
                for k in range(8):
                    for c in range(2):
                        nc.tensor.matmul(g1[:, 512 * c:512 * (c + 1)],
                                         h0T[:, k, :],
                                         w1[:, k, 512 * c:512 * (c + 1)],
                                         start=False, stop=(k == 7))

                # -- layer1 activations, cell update
                acts1 = work.tile([128, 1024], FP, tag="acts1")
                nc.scalar.activation(acts1[:, 0:512], g1[:, 0:512], AF.Sigmoid)
                nc.scalar.activation(acts1[:, 512:768], g1[:, 512:768], AF.Tanh)
                nc.scalar.activation(acts1[:, 768:1024], g1[:, 768:1024], AF.Sigmoid)
                c1n = work.tile([128, HC], FP, tag="c1")
                if t == 0:
                    nc.vector.tensor_mul(c1n[:], acts1[:, 0:256], acts1[:, 512:768])
                else:
                    tmp1 = work.tile([128, HC], FP, tag="tmp1")
                    nc.vector.tensor_mul(tmp1[:], acts1[:, 0:256], acts1[:, 512:768])
                    nc.vector.tensor_mul(c1n[:], acts1[:, 256:512], c1[:])
                    nc.vector.tensor_add(c1n[:], c1n[:], tmp1[:])
                c1 = c1n
                tc1 = work.tile([128, HC], FP, tag="tc1")
                nc.scalar.activation(tc1[:], c1[:], AF.Tanh)
                h1new = work.tile([128, HC], FP, tag="h1new")
                nc.vector.tensor_mul(h1new[:], acts1[:, 768:1024], tc1[:])

                if debug_step is not None and t == debug_step:
                    nc.sync.dma_start(dbg_d[0], h0new[:])
                    nc.sync.dma_start(dbg_d[1], h1new[:])
                    nc.sync.dma_start(dbg_d[2], c0[:])
                    nc.sync.dma_start(dbg_d[3], c1[:])
                    dg = work.tile([128, 1024], FP, tag="dbgg")
                    nc.vector.tensor_copy(dg[:], g0[:])
                    nc.sync.dma_start(dbg_d[4], dg[:, 0:256])
                    nc.sync.dma_start(dbg_d[5], acts0[:, 0:256])

                # -- fc partial (mask fc_m) rides AG2
                agin2 = dram.tile([257, 128], FP, tag="agin2")
                do_fc = 0 <= fc_m < 24 and not no_fc
                if do_fc:
                    fcout = work.tile([128, HC], FP, tag="fcout")
                    ypart = work.tile([128, 1], FP, tag="ypart")
                    nc.vector.tensor_mul(fcout[:], h1new[:], mask[:, fc_m, :])
                    nc.vector.tensor_reduce(ypart[:], fcout[:],
                                            mybir.AxisListType.X, OP.add)
                    # [128,1] partition column -> [1,128] DRAM row (tiny DMA)
                    nc.sync.dma_start(agin2[256:257, :], ypart[:])
                for s in range(2):
                    tp = tpp.tile([128, 128], FP, tag="tp")
                    nc.tensor.transpose(tp[:], h1new[:, 128 * s:128 * (s + 1)], eye[:])
                    st = stg.tile([128, 128], FP, tag="stage")
                    nc.vector.tensor_copy(st[:], tp[:])
                    nc.sync.dma_start(agin2[128 * s:128 * (s + 1), :], st[:])
                agout2 = dram.tile([4 * 257, 128], FP, tag="agout2")
                nc.gpsimd.collective_compute(
                    "AllGather", OP.bypass, replica_groups=GROUPS,
                    ins=[agin2[:]], outs=[agout2[:]])
                h1T = work.tile([128, 8, 128], FP, tag="h1T")
                for r in range(4):
                    for s in range(2):
                        base = 257 * r + 128 * s
                        nc.sync.dma_start(h1T[:, 2 * r + s, :],
                                          agout2[base:base + 128, :])
                agout2_prev = agout2

            # -- final y (m=23) assembled after the loop
            tmpy = work.tile([1, 4, 128], FP, tag="tmpy")
            for r in range(4):
                nc.sync.dma_start(tmpy[0:1, r, :],
                                  agout2_prev[257 * r + 256:257 * r + 257, :])
            ya = work.tile([1, 128], FP, tag="ya")
            nc.vector.tensor_add(ya[:], tmpy[0:1, 0, :], tmpy[0:1, 1, :])
            nc.vector.tensor_add(ya[:], ya[:], tmpy[0:1, 2, :])
            nc.vector.tensor_add(ya[:], ya[:], tmpy[0:1, 3, :])
            yfin = work.tile([1, 128], FP, tag="yfin")
            nc.vector.tensor_scalar(yfin[:], ya[:], float(fc_b), None, OP.add)
            m_fin = n_steps - (N_ENC - 1) - 1
            if 0 <= m_fin < 24:
                nc.sync.dma_start(out_d[m_fin:m_fin + 1, :], yfin[:])

    nc.compile()
    return nc


def _host_masks():
    """Dropout masks folded with fc_w/KEEP are data-independent except for fc_w;
    computed in kernel() where fc_w is available."""
    import jax
    with jax.default_device(jax.devices("cpu")[0]):
        dkey = jax.random.key(42)
        ms = []
        for t in range(24):
            m = jax.random.bernoulli(jax.random.fold_in(dkey, t), KEEP, (B_FULL, H))
            ms.append(np.asarray(m))
    return np.stack(ms)  # [24, 256, 1024] bool


PROFILE = False       # set True (e.g. from test.py) to capture an NTFF trace
LAST_RESULT = None    # BassKernelResults of the last kernel() call
LAST_NC = None        # compiled Bass module of the last kernel() call
LAST_IN_MAPS = None   # per-core input maps of the last kernel() call


def kernel(x, conv_w, conv_b, Wih0, Whh0, bih0, bhh0,
           Wih1, Whh1, bih1, bhh1, fc_w, fc_b):
    global LAST_RESULT, LAST_NC, LAST_IN_MAPS
    from concourse.bass_utils import run_bass_kernel_spmd

    x = np.asarray(x, np.float32)
    conv_w = np.asarray(conv_w, np.float32)
    conv_b = np.asarray(conv_b, np.float32)
    Wih0 = np.asarray(Wih0, np.float32); Whh0 = np.asarray(Whh0, np.float32)
    Wih1 = np.asarray(Wih1, np.float32); Whh1 = np.asarray(Whh1, np.float32)
    b0 = np.asarray(bih0, np.float32) + np.asarray(bhh0, np.float32)
    b1 = np.asarray(bih1, np.float32) + np.asarray(bhh1, np.float32)
    fc_w = np.asarray(fc_w, np.float32); fc_b = np.asarray(fc_b, np.float32)

    bern = _host_masks()                               # [24, 256, 1024]
    M = bern.astype(np.float32) * (fc_w[0][None, None, :] / KEEP)

    nc = _build(conv_w, conv_b, float(fc_b[0]))

    eye = np.eye(128, dtype=np.float32)
    in_maps = []
    for core in range(8):
        g, r = core // 4, core % 4
        rows = np.concatenate([1024 * q + 256 * r + np.arange(256) for q in range(4)])
        w0c = np.ascontiguousarray(Whh0[rows, :].T)            # [1024, 1024]
        xb0 = np.stack([Wih0[rows, 0], b0[rows]])              # [2, 1024]
        w1c = np.concatenate([Wih1[rows, :].T, Whh1[rows, :].T])  # [2048, 1024]
        in_maps.append({
            "x": np.ascontiguousarray(x[128 * g:128 * (g + 1), :WIN_IN, :]),
            "w0": w0c,
            "xb0": xb0,
            "w1": w1c,
            "b1": b1[rows][None, :],
            "mask": np.ascontiguousarray(
                M[:, 128 * g:128 * (g + 1), 256 * r:256 * (r + 1)]),
            "eye": eye,
        })

    res = run_bass_kernel_spmd(nc, in_maps, core_ids=list(range(8)),
                               trace=PROFILE)
    LAST_RESULT = res
    LAST_NC = nc
    LAST_IN_MAPS = in_maps
    y0 = res.results[0]["out"].T                                # [128, 24]
    y1 = res.results[4]["out"].T
    return np.concatenate([y0, y1], axis=0).astype(np.float32)  # [256, 24]


# revision 22
# speedup vs baseline: 1.3633x; 1.3633x over previous
"""Trainium2 Bass kernel for nn_BayesianLSTM (2-layer LSTM H=1024, conv front-end,
84 encoder + 23 decoder steps, B=256, fp32).

Sharding: 2 data-parallel groups (batch 128 each) x 4-way model parallel within
each group (each core owns a 256-wide slice of the hidden dim of both layers).
Recurrent matmuls are batch-stationary (stationary = h^T tile [K=128, B=128],
moving = weight chunks [K=128, N=512]), giving full PE-array utilization.
Hidden-state chunks are exchanged once per layer per step with an intra-group
AllGather of the transposed chunk (h^T layout feeds the next step's stationary
operand directly). Dropout masks (fixed seed, data-independent) are precomputed
on host and folded with fc_w; the fc dot-product partials ride the layer-1
AllGather.
"""
import numpy as np

H = 1024
B_FULL = 256
B = 128           # batch per core (per DP group)
HC = 256          # hidden chunk per core
WIN_IN, WIN_OUT = 168, 24
N_ENC = 84
N_STEPS = N_ENC + WIN_OUT - 1   # 107 cell evaluations
KEEP = 0.8
FP32 = None       # set after imports


def _build(conv_w, conv_b, fc_b, n_steps=N_STEPS, debug_step=None, no_fc=False):
    import concourse.bass as bass
    import concourse.tile as tile
    from concourse import bacc, mybir

    FP = mybir.dt.float32
    AF = mybir.ActivationFunctionType
    OP = mybir.AluOpType

    nc = bacc.Bacc(None, target_bir_lowering=False, num_devices=8)

    # ---- per-core external tensors ----
    x_d = nc.dram_tensor("x", [B, WIN_IN, 8], FP, kind="ExternalInput")
    w0_d = nc.dram_tensor("w0", [H, 1024], FP, kind="ExternalInput")     # Whh0^T slice
    xb0_d = nc.dram_tensor("xb0", [2, 1024], FP, kind="ExternalInput")   # [Wih0 col; b0]
    w1_d = nc.dram_tensor("w1", [2 * H, 1024], FP, kind="ExternalInput") # [Wih1^T; Whh1^T]
    b1_d = nc.dram_tensor("b1", [1, 1024], FP, kind="ExternalInput")
    mask_d = nc.dram_tensor("mask", [24, B, HC], FP, kind="ExternalInput")
    eye_d = nc.dram_tensor("eye", [128, 128], FP, kind="ExternalInput")
    out_d = nc.dram_tensor("out", [24, B], FP, kind="ExternalOutput")
    if debug_step is not None:
        dbg_d = nc.dram_tensor("dbg", [6, B, HC], FP, kind="ExternalOutput")

    GROUPS = [[0, 1, 2, 3], [4, 5, 6, 7]]

    with tile.TileContext(nc) as tc:
        with tc.tile_pool(name="const", bufs=1) as constp, \
             tc.tile_pool(name="work", bufs=2) as work, \
             tc.tile_pool(name="xbp", bufs=3) as xbp, \
             tc.tile_pool(name="stg", bufs=6) as stg, \
             tc.tile_pool(name="ps", bufs=1, space="PSUM") as ps, \
             tc.tile_pool(name="tpp", bufs=4, space="PSUM") as tpp, \
             tc.tile_pool(name="dram", bufs=2, space="DRAM") as dram:

            # ---- load constants ----
            w0 = constp.tile([128, 8, 1024], FP, tag="w0")
            for k in range(8):
                nc.sync.dma_start(w0[:, k, :], w0_d[128 * k:128 * (k + 1), :])
            w1 = constp.tile([128, 16, 1024], FP, tag="w1")
            for k in range(16):
                nc.sync.dma_start(w1[:, k, :], w1_d[128 * k:128 * (k + 1), :])
            xb0w = constp.tile([2, 1024], FP, tag="xb0w")
            nc.sync.dma_start(xb0w[:], xb0_d[:])
            b1 = constp.tile([1, 1024], FP, tag="b1")
            nc.sync.dma_start(b1[:], b1_d[:])
            mask = constp.tile([128, 24, HC], FP, tag="mask")
            for m in range(24):
                nc.sync.dma_start(mask[:, m, :], mask_d[m])
            eye = constp.tile([128, 128], FP, tag="eye")
            nc.sync.dma_start(eye[:], eye_d[:])
            ones = constp.tile([1, 128], FP, tag="ones")
            nc.vector.memset(ones[:], 1.0)

            # ---- conv front-end (per-core batch half) ----
            xpad = constp.tile([128, WIN_IN + 2, 8], FP, tag="xpad")
            nc.vector.memset(xpad[:], 0.0)
            nc.sync.dma_start(xpad[:, 1:WIN_IN + 1, :], x_d[:])
            acc = constp.tile([128, WIN_IN], FP, tag="cacc")
            tmpc = constp.tile([128, WIN_IN], FP, tag="ctmp")
            first = True
            for f in range(8):
                for k in range(3):
                    wv = float(conv_w[0, f, k])
                    src = xpad[:, k:k + WIN_IN, f]
                    if first:
                        nc.vector.tensor_scalar(acc[:], src, wv, None, OP.mult)
                        first = False
                    else:
                        nc.vector.tensor_scalar(tmpc[:], src, wv, None, OP.mult)
                        nc.vector.tensor_add(acc[:], acc[:], tmpc[:])
            relu = constp.tile([128, WIN_IN], FP, tag="crelu")
            nc.scalar.activation(relu[:], acc[:], AF.Relu, bias=float(conv_b[0]))
            pool = constp.tile([128, N_ENC], FP, tag="cpool")
            nc.vector.tensor_max(pool[:], relu[:, 0:WIN_IN:2], relu[:, 1:WIN_IN:2])
            seq_ps = tpp.tile([128, 128], FP, tag="tp")
            nc.tensor.transpose(seq_ps[0:N_ENC, :], pool[:], eye[:])
            seqT = constp.tile([N_ENC, 128], FP, tag="seqT")
            nc.vector.tensor_copy(seqT[:], seq_ps[0:N_ENC, :])

            # ---- recurrent steps ----
            h0T = h1T = None          # [128, 8, 128] stationary h^T tiles
            c0 = c1 = None            # [128, 256]
            agout2_prev = None
            agin_cur = None           # rolling merged-AG input [4,128,128]

            for t in range(n_steps):
                fc_m = t - (N_ENC - 1)   # mask/output index produced at THIS step's tail
                dec_m = t - N_ENC        # y index consumed at this step (decoder)

                # -- xb stationary tile [2,128]: row0 = x_t (enc) or y (dec), row1 = 1
                xb = xbp.tile([2, 128], FP, tag="xb")
                nc.vector.memset(xb[:], 1.0)
                if t < N_ENC:
                    nc.sync.dma_start(xb[0:1, :], seqT[t:t + 1, :])
                else:
                    tmpy = work.tile([1, 4, 128], FP, tag="tmpy")
                    for r in range(4):
                        nc.sync.dma_start(tmpy[0:1, r, :],
                                          agout2_prev[257 * r + 256:257 * r + 257, :])
                    ya = work.tile([1, 128], FP, tag="ya")
                    nc.vector.tensor_add(ya[:], tmpy[0:1, 0, :], tmpy[0:1, 1, :])
                    nc.vector.tensor_add(ya[:], ya[:], tmpy[0:1, 2, :])
                    nc.vector.tensor_add(ya[:], ya[:], tmpy[0:1, 3, :])
                    nc.vector.tensor_scalar(xb[0:1, :], ya[:], float(fc_b), None, OP.add)
                    nc.sync.dma_start(out_d[dec_m:dec_m + 1, :], xb[0:1, :])

                # PE queue is in-order, so emission order = execution order.
                # Encoder: emit the h0new transposes BEFORE the Whh1 block so
                # AllGather#1 launches ~4us earlier; Whh1 then covers the
                # gather. Decoder: emit Whh1 right after the g0 h-part so the
                # PE has cover while the y-feedback gather (AG#2 of t-1)
                # completes; the xb matmul (which waits on it) comes after.
                g0 = ps.tile([128, 1024], FP, tag="g0")
                g1 = ps.tile([128, 1024], FP, tag="g1")

                def g0_h_part():
                    if t > 0:
                        for k in range(8):
                            for c in range(2):
                                nc.tensor.matmul(g0[:, 512 * c:512 * (c + 1)],
                                                 h0T[:, k, :],
                                                 w0[:, k, 512 * c:512 * (c + 1)],
                                                 start=(k == 0), stop=False)

                def g1_bias_whh1():
                    for c in range(2):
                        nc.tensor.matmul(g1[:, 512 * c:512 * (c + 1)], ones[:],
                                         b1[:, 512 * c:512 * (c + 1)],
                                         start=True, stop=False)
                    if t > 0:
                        for k in range(8):
                            for c in range(2):
                                nc.tensor.matmul(g1[:, 512 * c:512 * (c + 1)],
                                                 h1T[:, k, :],
                                                 w1[:, 8 + k, 512 * c:512 * (c + 1)],
                                                 start=False, stop=False)

                def g0_xb_part():
                    for c in range(2):
                        nc.tensor.matmul(g0[:, 512 * c:512 * (c + 1)], xb[:],
                                         xb0w[:, 512 * c:512 * (c + 1)],
                                         start=(t == 0), stop=True)

                enc_order = t < N_ENC
                g0_h_part()
                if not enc_order:
                    g1_bias_whh1()
                g0_xb_part()

                # -- layer0 activations, cell update
                acts0 = work.tile([128, 1024], FP, tag="acts0")
                nc.scalar.activation(acts0[:, 0:512], g0[:, 0:512], AF.Sigmoid)
                nc.scalar.activation(acts0[:, 512:768], g0[:, 512:768], AF.Tanh)
                nc.scalar.activation(acts0[:, 768:1024], g0[:, 768:1024], AF.Sigmoid)
                c0n = work.tile([128, HC], FP, tag="c0")
                if t == 0:
                    nc.vector.tensor_mul(c0n[:], acts0[:, 0:256], acts0[:, 512:768])
                else:
                    tmp0 = work.tile([128, HC], FP, tag="tmp0")
                    nc.vector.tensor_mul(tmp0[:], acts0[:, 0:256], acts0[:, 512:768])
                    nc.vector.tensor_mul(c0n[:], acts0[:, 256:512], c0[:])
                    nc.vector.tensor_add(c0n[:], c0n[:], tmp0[:])
                c0 = c0n
                tc0 = work.tile([128, HC], FP, tag="tc0")
                nc.scalar.activation(tc0[:], c0[:], AF.Tanh)
                h0new = work.tile([128, HC], FP, tag="h0new")
                nc.vector.tensor_mul(h0new[:], acts0[:, 768:1024], tc0[:])

                # -- transpose h0new chunk, AllGather within group
                if enc_order:
                    # Merged gather: rows 0-1 = h0newT(t), rows 2-3 = h1newT(t-1)
                    # (written at the previous step's tail). One collective per
                    # encoder step instead of two — the collective queue is the
                    # serial bottleneck.
                    if agin_cur is None:
                        agin_cur = dram.tile([4, 128, 128], FP, tag="agin1")
                    for s in range(2):
                        tp = tpp.tile([128, 128], FP, tag="tp")
                        nc.tensor.transpose(tp[:], h0new[:, 128 * s:128 * (s + 1)], eye[:])
                        st = stg.tile([128, 128], FP, tag="stage")
                        nc.vector.tensor_copy(st[:], tp[:])
                        nc.sync.dma_start(agin_cur[s], st[:])
                    agout1 = dram.tile([16, 128, 128], FP, tag="agout1")
                    nc.gpsimd.collective_compute(
                        "AllGather", OP.bypass, replica_groups=GROUPS,
                        ins=[agin_cur[:]], outs=[agout1[:]])
                    h0T = work.tile([128, 8, 128], FP, tag="h0T")
                    for r in range(4):
                        for s in range(2):
                            nc.sync.dma_start(h0T[:, 2 * r + s, :], agout1[4 * r + s])
                    if t > 0:
                        h1T = work.tile([128, 8, 128], FP, tag="h1T")
                        for r in range(4):
                            for s in range(2):
                                nc.sync.dma_start(h1T[:, 2 * r + s, :],
                                                  agout1[4 * r + 2 + s])
                    g1_bias_whh1()
                else:
                    agin1 = dram.tile([2, 128, 128], FP, tag="agin1")
                    for s in range(2):
                        tp = tpp.tile([128, 128], FP, tag="tp")
                        nc.tensor.transpose(tp[:], h0new[:, 128 * s:128 * (s + 1)], eye[:])
                        st = stg.tile([128, 128], FP, tag="stage")
                        nc.vector.tensor_copy(st[:], tp[:])
                        nc.sync.dma_start(agin1[s], st[:])
                    agout1 = dram.tile([8, 128, 128], FP, tag="agout1")
                    nc.gpsimd.collective_compute(
                        "AllGather", OP.bypass, replica_groups=GROUPS,
                        ins=[agin1[:]], outs=[agout1[:]])
                    h0T = work.tile([128, 8, 128], FP, tag="h0T")
                    for k in range(8):
                        nc.sync.dma_start(h0T[:, k, :], agout1[k])

                # -- layer1 gates part 2: h0new @ Wih1^T
                for k in range(8):
                    for c in range(2):
                        nc.tensor.matmul(g1[:, 512 * c:512 * (c + 1)],
                                         h0T[:, k, :],
                                         w1[:, k, 512 * c:512 * (c + 1)],
                                         start=False, stop=(k == 7))

                # -- layer1 activations, cell update
                acts1 = work.tile([128, 1024], FP, tag="acts1")
                nc.scalar.activation(acts1[:, 0:512], g1[:, 0:512], AF.Sigmoid)
                nc.scalar.activation(acts1[:, 512:768], g1[:, 512:768], AF.Tanh)
                nc.scalar.activation(acts1[:, 768:1024], g1[:, 768:1024], AF.Sigmoid)
                c1n = work.tile([128, HC], FP, tag="c1")
                if t == 0:
                    nc.vector.tensor_mul(c1n[:], acts1[:, 0:256], acts1[:, 512:768])
                else:
                    tmp1 = work.tile([128, HC], FP, tag="tmp1")
                    nc.vector.tensor_mul(tmp1[:], acts1[:, 0:256], acts1[:, 512:768])
                    nc.vector.tensor_mul(c1n[:], acts1[:, 256:512], c1[:])
                    nc.vector.tensor_add(c1n[:], c1n[:], tmp1[:])
                c1 = c1n
                tc1 = work.tile([128, HC], FP, tag="tc1")
                nc.scalar.activation(tc1[:], c1[:], AF.Tanh)
                h1new = work.tile([128, HC], FP, tag="h1new")
                nc.vector.tensor_mul(h1new[:], acts1[:, 768:1024], tc1[:])

                if debug_step is not None and t == debug_step:
                    nc.sync.dma_start(dbg_d[0], h0new[:])
                    nc.sync.dma_start(dbg_d[1], h1new[:])
                    nc.sync.dma_start(dbg_d[2], c0[:])
                    nc.sync.dma_start(dbg_d[3], c1[:])
                    dg = work.tile([128, 1024], FP, tag="dbgg")
                    nc.vector.tensor_copy(dg[:], g0[:])
                    nc.sync.dma_start(dbg_d[4], dg[:, 0:256])
                    nc.sync.dma_start(dbg_d[5], acts0[:, 0:256])

                if enc_order and t < N_ENC - 1:
                    # Merged mode: stage h1newT(t) into the NEXT step's gather
                    # input (rows 2-3); no second collective this step.
                    agin_next = dram.tile([4, 128, 128], FP, tag="agin1")
                    for s in range(2):
                        tp = tpp.tile([128, 128], FP, tag="tp")
                        nc.tensor.transpose(tp[:], h1new[:, 128 * s:128 * (s + 1)],
                                            eye[:])
                        st = stg.tile([128, 128], FP, tag="stage")
                        nc.vector.tensor_copy(st[:], tp[:])
                        nc.sync.dma_start(agin_next[2 + s], st[:])
                    agin_cur = agin_next
                    continue

                # -- fc partial (mask fc_m) rides AG2 (step 83 + decoder)
                agin2 = dram.tile([257, 128], FP, tag="agin2")
                do_fc = 0 <= fc_m < 24 and not no_fc
                if do_fc:
                    fcout = work.tile([128, HC], FP, tag="fcout")
                    ypart = work.tile([128, 1], FP, tag="ypart")
                    nc.vector.tensor_mul(fcout[:], h1new[:], mask[:, fc_m, :])
                    nc.vector.tensor_reduce(ypart[:], fcout[:],
                                            mybir.AxisListType.X, OP.add)
                    # [128,1] partition column -> [1,128] DRAM row (tiny DMA)
                    nc.sync.dma_start(agin2[256:257, :], ypart[:])
                for s in range(2):
                    tp = tpp.tile([128, 128], FP, tag="tp")
                    nc.tensor.transpose(tp[:], h1new[:, 128 * s:128 * (s + 1)], eye[:])
                    st = stg.tile([128, 128], FP, tag="stage")
                    nc.vector.tensor_copy(st[:], tp[:])
                    nc.sync.dma_start(agin2[128 * s:128 * (s + 1), :], st[:])
                agout2 = dram.tile([4 * 257, 128], FP, tag="agout2")
                nc.gpsimd.collective_compute(
                    "AllGather", OP.bypass, replica_groups=GROUPS,
                    ins=[agin2[:]], outs=[agout2[:]])
                h1T = work.tile([128, 8, 128], FP, tag="h1T")
                for r in range(4):
                    for s in range(2):
                        base = 257 * r + 128 * s
                        nc.sync.dma_start(h1T[:, 2 * r + s, :],
                                          agout2[base:base + 128, :])
                agout2_prev = agout2

            # -- final y (m=23) assembled after the loop
            if agout2_prev is None:
                agout2_prev = dram.tile([4 * 257, 128], FP, tag="agout2")
            tmpy = work.tile([1, 4, 128], FP, tag="tmpy")
            for r in range(4):
                nc.sync.dma_start(tmpy[0:1, r, :],
                                  agout2_prev[257 * r + 256:257 * r + 257, :])
            ya = work.tile([1, 128], FP, tag="ya")
            nc.vector.tensor_add(ya[:], tmpy[0:1, 0, :], tmpy[0:1, 1, :])
            nc.vector.tensor_add(ya[:], ya[:], tmpy[0:1, 2, :])
            nc.vector.tensor_add(ya[:], ya[:], tmpy[0:1, 3, :])
            yfin = work.tile([1, 128], FP, tag="yfin")
            nc.vector.tensor_scalar(yfin[:], ya[:], float(fc_b), None, OP.add)
            m_fin = n_steps - (N_ENC - 1) - 1
            if 0 <= m_fin < 24:
                nc.sync.dma_start(out_d[m_fin:m_fin + 1, :], yfin[:])

    nc.compile()
    return nc


def _host_masks():
    """Dropout masks folded with fc_w/KEEP are data-independent except for fc_w;
    computed in kernel() where fc_w is available."""
    import jax
    with jax.default_device(jax.devices("cpu")[0]):
        dkey = jax.random.key(42)
        ms = []
        for t in range(24):
            m = jax.random.bernoulli(jax.random.fold_in(dkey, t), KEEP, (B_FULL, H))
            ms.append(np.asarray(m))
    return np.stack(ms)  # [24, 256, 1024] bool


PROFILE = False       # set True (e.g. from test.py) to capture an NTFF trace
LAST_RESULT = None    # BassKernelResults of the last kernel() call
LAST_NC = None        # compiled Bass module of the last kernel() call
LAST_IN_MAPS = None   # per-core input maps of the last kernel() call


def kernel(x, conv_w, conv_b, Wih0, Whh0, bih0, bhh0,
           Wih1, Whh1, bih1, bhh1, fc_w, fc_b):
    global LAST_RESULT, LAST_NC, LAST_IN_MAPS
    from concourse.bass_utils import run_bass_kernel_spmd

    x = np.asarray(x, np.float32)
    conv_w = np.asarray(conv_w, np.float32)
    conv_b = np.asarray(conv_b, np.float32)
    Wih0 = np.asarray(Wih0, np.float32); Whh0 = np.asarray(Whh0, np.float32)
    Wih1 = np.asarray(Wih1, np.float32); Whh1 = np.asarray(Whh1, np.float32)
    b0 = np.asarray(bih0, np.float32) + np.asarray(bhh0, np.float32)
    b1 = np.asarray(bih1, np.float32) + np.asarray(bhh1, np.float32)
    fc_w = np.asarray(fc_w, np.float32); fc_b = np.asarray(fc_b, np.float32)

    bern = _host_masks()                               # [24, 256, 1024]
    M = bern.astype(np.float32) * (fc_w[0][None, None, :] / KEEP)

    nc = _build(conv_w, conv_b, float(fc_b[0]))

    eye = np.eye(128, dtype=np.float32)
    in_maps = []
    for core in range(8):
        g, r = core // 4, core % 4
        rows = np.concatenate([1024 * q + 256 * r + np.arange(256) for q in range(4)])
        w0c = np.ascontiguousarray(Whh0[rows, :].T)            # [1024, 1024]
        xb0 = np.stack([Wih0[rows, 0], b0[rows]])              # [2, 1024]
        w1c = np.concatenate([Wih1[rows, :].T, Whh1[rows, :].T])  # [2048, 1024]
        in_maps.append({
            "x": np.ascontiguousarray(x[128 * g:128 * (g + 1), :WIN_IN, :]),
            "w0": w0c,
            "xb0": xb0,
            "w1": w1c,
            "b1": b1[rows][None, :],
            "mask": np.ascontiguousarray(
                M[:, 128 * g:128 * (g + 1), 256 * r:256 * (r + 1)]),
            "eye": eye,
        })

    res = run_bass_kernel_spmd(nc, in_maps, core_ids=list(range(8)),
                               trace=PROFILE)
    LAST_RESULT = res
    LAST_NC = nc
    LAST_IN_MAPS = in_maps
    y0 = res.results[0]["out"].T                                # [128, 24]
    y1 = res.results[4]["out"].T
    return np.concatenate([y0, y1], axis=0).astype(np.float32)  # [256, 24]
